# revision 14
# baseline (speedup 1.0000x reference)
"""Trainium2 Bass kernel for fused MHA block (proj + attention + dense + residual + LN).

NOTE: the reference reshapes attn [B,NH,S,D] -> [B,S,HID] WITHOUT transposing heads
back. Hence output row s' depends only on head h' = s'//128 evaluated at queries
q' = 16*(s'%128) + m, m in [0,16). This makes a (batch, head-group) sharding fully
local: core c -> batch b = c//4, head group g = c%4 (heads 4g..4g+3, output rows
512g..512g+512). No collectives, no replicated work.

Device layout (all matmuls float32r = full PE rate at N>=256):
  - host pre-transposes q,k,v to [HID, S]; q/k/v weights are column-sliced per
    head group on the host.
  - hkT_g = wkg.T @ kT  [256, S]   resident ; hqT_g = wqg.T @ qT [256, S] resident
  - hv_g  = vT.T @ wvg  [S, 256]   resident, with a ones column per head (softmax
    denominator rides along the PV matmul)
  - per (head, 512-query-block): logitsT[sk, q] = hk_h.T @ hq_h ;
    expw = exp(scale*logitsT + maskbias[sk])  (mask bias is per-partition here;
    no max subtraction needed at these magnitudes)
  - PV with m-major strided rhs: attnT psum columns ordered (m, s') so the
    scrambled-reshape layout falls out naturally; normalize by PE-broadcast
    reciprocal denominators while scattering into y tiles [128, 8, 128].
  - dense: out[s', o] = y.T @ wd (+bd) ; + residual q ; LayerNorm along free dim.
"""

import os
import numpy as np

B, S, HID, NH, D = 2, 2048, 1024, 16, 64
GH = 4            # heads per core
GW = GH * D       # 256 hidden cols per group
QR = 512          # output rows per core
SCALE = 0.125     # 1/sqrt(D)
LN_EPS = 1e-12
NCORES = 8

_cache = {}
last_results = None


def _build_nc():
    from contextlib import ExitStack
    import concourse.bass as bass
    import concourse.bacc as bacc
    import concourse.tile as tile
    from concourse import mybir

    f32 = mybir.dt.float32
    f32r = mybir.dt.float32r
    AF = mybir.ActivationFunctionType
    ALU = mybir.AluOpType

    nc = bacc.Bacc("TRN2", target_bir_lowering=False, debug=False)

    kT_d = nc.dram_tensor("kT", [HID, S], f32r, kind="ExternalInput").ap()
    vT_d = nc.dram_tensor("vT", [HID, S], f32r, kind="ExternalInput").ap()
    qT_d = nc.dram_tensor("qT", [HID, S], f32r, kind="ExternalInput").ap()
    qres_d = nc.dram_tensor("qres", [QR, HID], f32, kind="ExternalInput").ap()
    mb_d = nc.dram_tensor("mbias", [128, 16], f32, kind="ExternalInput").ap()
    wkg_d = nc.dram_tensor("wkg", [HID, GW], f32r, kind="ExternalInput").ap()
    wvg_d = nc.dram_tensor("wvg", [HID, GW], f32r, kind="ExternalInput").ap()
    wqg_d = nc.dram_tensor("wqg", [HID, GW], f32r, kind="ExternalInput").ap()
    wd_d = nc.dram_tensor("wd", [HID, HID], f32r, kind="ExternalInput").ap()
    bkg_d = nc.dram_tensor("bkg", [128, 2], f32, kind="ExternalInput").ap()
    bqg_d = nc.dram_tensor("bqg", [128, 2], f32, kind="ExternalInput").ap()
    bvg_d = nc.dram_tensor("bvg", [1, GW], f32r, kind="ExternalInput").ap()
    bd_d = nc.dram_tensor("bd_row", [1, HID], f32r, kind="ExternalInput").ap()
    gamma_d = nc.dram_tensor("gamma", [1, HID], f32, kind="ExternalInput").ap()
    beta_d = nc.dram_tensor("beta", [1, HID], f32, kind="ExternalInput").ap()
    ones_row_d = nc.dram_tensor("ones_row", [1, 128], f32r, kind="ExternalInput").ap()
    ones_col_d = nc.dram_tensor("ones_col", [128, GH, 1], f32r, kind="ExternalInput").ap()
    out_d = nc.dram_tensor("out", [QR, HID], f32, kind="ExternalOutput").ap()

    def bcast_ap(src, parts):
        # replicate a [1, N] DRAM row across `parts` partitions (0-stride read)
        return bass.AP(tensor=src.tensor, offset=src.offset,
                       ap=[[0, parts]] + list(src.ap[1:]))

    with ExitStack() as ctx:
        tc = ctx.enter_context(tile.TileContext(nc))

        wpool = ctx.enter_context(tc.tile_pool(name="wmat", bufs=8))
        wdpool = ctx.enter_context(tc.tile_pool(name="wdp", bufs=8))
        instr = ctx.enter_context(tc.tile_pool(name="instream", bufs=8))
        ps = ctx.enter_context(tc.tile_pool(name="ps", bufs=8, space="PSUM"))
        hkpool = ctx.enter_context(tc.tile_pool(name="hk", bufs=2))
        hqpool = ctx.enter_context(tc.tile_pool(name="hq", bufs=2))
        hvpool = ctx.enter_context(tc.tile_pool(name="hv", bufs=16))
        expwp = ctx.enter_context(tc.tile_pool(name="expw", bufs=16))
        ypool = ctx.enter_context(tc.tile_pool(name="y", bufs=4))
        xpool = ctx.enter_context(tc.tile_pool(name="xln", bufs=2))
        iopool = ctx.enter_context(tc.tile_pool(name="io", bufs=2))
        qrpool = ctx.enter_context(tc.tile_pool(name="qrp", bufs=4))
        cpool = ctx.enter_context(tc.tile_pool(name="const", bufs=1))
        small = ctx.enter_context(tc.tile_pool(name="small", bufs=2))

        # ---- constants ----
        mb_sb = cpool.tile([128, 16], f32, tag="mb")
        nc.sync.dma_start(mb_sb, mb_d)
        bkg_sb = cpool.tile([128, 2], f32, tag="bkg")
        nc.sync.dma_start(bkg_sb, bkg_d)
        bqg_sb = cpool.tile([128, 2], f32, tag="bqg")
        nc.sync.dma_start(bqg_sb, bqg_d)
        bvg_sb = cpool.tile([1, GW], f32r, tag="bvg")
        nc.sync.dma_start(bvg_sb, bvg_d)
        bd_sb = cpool.tile([1, HID], f32r, tag="bd")
        nc.sync.dma_start(bd_sb, bd_d)
        gamma_sb = cpool.tile([128, HID], f32, tag="gamma")
        nc.sync.dma_start(gamma_sb, bcast_ap(gamma_d, 128))
        beta_sb = cpool.tile([128, HID], f32, tag="beta")
        nc.sync.dma_start(beta_sb, bcast_ap(beta_d, 128))
        ones = cpool.tile([1, 128], f32r, tag="ones")
        nc.sync.dma_start(ones, ones_row_d)
        eps_sb = cpool.tile([128, 1], f32, tag="eps")
        nc.vector.memset(eps_sb, LN_EPS)

        # ---- stage A: hkT_g = wkg.T @ kT + bk -> resident [2][128, S] ----
        wkg_sb = [wpool.tile([128, GW], f32r, tag="wg", name=f"wkg_sb{i}")
                  for i in range(8)]
        for ic in range(8):
            nc.sync.dma_start(wkg_sb[ic], wkg_d[ic * 128:(ic + 1) * 128, :])
        hk_sb = [hkpool.tile([128, S], f32r, tag="hk", name=f"hk_sb{i}")
                 for i in range(2)]
        for nb in range(4):
            psA = [ps.tile([128, 512], f32, tag="ps", name=f"psA{nb}_{i}")
                   for i in range(2)]
            for ic in range(8):
                kp = instr.tile([128, 512], f32r, tag="xin", name=f"kp{nb}_{ic}")
                nc.sync.dma_start(kp, kT_d[ic * 128:(ic + 1) * 128,
                                           nb * 512:(nb + 1) * 512])
                for oc in range(2):
                    nc.tensor.matmul(psA[oc],
                                     wkg_sb[ic][:, oc * 128:(oc + 1) * 128],
                                     kp,
                                     start=(ic == 0), stop=(ic == 7))
            for oc in range(2):
                nc.scalar.activation(hk_sb[oc][:, nb * 512:(nb + 1) * 512],
                                     psA[oc], AF.Identity,
                                     bias=bkg_sb[:, oc:oc + 1], scale=1.0)

        # ---- stage Q: hqT_g = wqg.T @ qT + bq -> resident [2][128, S] ----
        wqg_sb = [wpool.tile([128, GW], f32r, tag="wg", name=f"wqg_sb{i}")
                  for i in range(8)]
        for ic in range(8):
            nc.sync.dma_start(wqg_sb[ic], wqg_d[ic * 128:(ic + 1) * 128, :])
        hq_sb = [hqpool.tile([128, S], f32r, tag="hq", name=f"hq_sb{i}")
                 for i in range(2)]
        for nb in range(4):
            psQ = [ps.tile([128, 512], f32, tag="ps", name=f"psQ{nb}_{i}")
                   for i in range(2)]
            for ic in range(8):
                qp = instr.tile([128, 512], f32r, tag="xin", name=f"qp{nb}_{ic}")
                nc.sync.dma_start(qp, qT_d[ic * 128:(ic + 1) * 128,
                                           nb * 512:(nb + 1) * 512])
                for oc in range(2):
                    nc.tensor.matmul(psQ[oc],
                                     wqg_sb[ic][:, oc * 128:(oc + 1) * 128],
                                     qp,
                                     start=(ic == 0), stop=(ic == 7))
            for oc in range(2):
                nc.scalar.activation(hq_sb[oc][:, nb * 512:(nb + 1) * 512],
                                     psQ[oc], AF.Identity,
                                     bias=bqg_sb[:, oc:oc + 1], scale=1.0)

        # ---- stage B: hv_g = vT.T @ wvg + bv -> resident [16][128, 4, 65] ----
        wvg_sb = [wpool.tile([128, GW], f32r, tag="wg", name=f"wvg_sb{i}")
                  for i in range(8)]
        for ic in range(8):
            nc.sync.dma_start(wvg_sb[ic], wvg_d[ic * 128:(ic + 1) * 128, :])
        hv_sb = [hvpool.tile([128, GH, D + 1], f32r, tag="hv", name=f"hv_sb{i}")
                 for i in range(16)]
        for sc in range(16):
            psB = ps.tile([128, GW], f32, tag="ps", name=f"psB{sc}")
            for ic in range(8):
                vp = instr.tile([128, 128], f32r, tag="xin", name=f"vp{sc}_{ic}")
                nc.sync.dma_start(vp, vT_d[ic * 128:(ic + 1) * 128,
                                           sc * 128:(sc + 1) * 128])
                nc.tensor.matmul(psB, vp, wvg_sb[ic],
                                 start=(ic == 0), stop=False)
            nc.tensor.matmul(psB, ones, bvg_sb,
                             start=False, stop=True)
            nc.vector.tensor_copy(hv_sb[sc][:, :, 0:D],
                                  psB.rearrange("p (h d) -> p h d", h=GH))
            nc.sync.dma_start(hv_sb[sc][:, :, D:D + 1], ones_col_d)

        # ---- stage C: attention, heads x 512-query blocks ----
        # y[j] tiles: [128, 8, 128]: partition = (m%2)*64 + d, mid = m//2, free = s'
        y_sb = [ypool.tile([128, 8, 128], f32r, tag="y", name=f"y_sb{i}")
                for i in range(GH)]
        for h in range(GH):
            hc, half = h // 2, h % 2
            lo, hi = half * 64, half * 64 + 64
            for qn in range(4):
                at_ps = ps.tile([D + 1, 512], f32, tag="ps",
                                name=f"at_ps{h}_{qn}")
                ews = []
                for sk in range(16):
                    lg = ps.tile([128, 512], f32, tag="ps",
                                 name=f"lg{h}_{qn}_{sk}")
                    nc.tensor.matmul(lg,
                                     hk_sb[hc][lo:hi, sk * 128:(sk + 1) * 128],
                                     hq_sb[hc][lo:hi, qn * 512:(qn + 1) * 512],
                                     start=True, stop=True)
                    ew = expwp.tile([128, 512], f32r, tag="expw",
                                    name=f"ew{h}_{qn}_{sk}")
                    nc.scalar.activation(ew, lg, AF.Exp,
                                         bias=mb_sb[:, sk:sk + 1], scale=SCALE)
                    ews.append(ew)
                for sk in range(16):
                    # m-major column order: n = m*32 + s''  (q_local = 16*s'' + m)
                    rhs = ews[sk].rearrange("p (s m) -> p m s", m=16)
                    nc.tensor.matmul(at_ps, hv_sb[sk][:, h, :],
                                     rhs,
                                     start=(sk == 0), stop=(sk == 15))

                den = small.tile([1, 512], f32r, tag="den", name=f"den{h}_{qn}")
                with nc.allow_low_precision(reason="f32r is 32-bit storage"):
                    nc.vector.reciprocal(den, at_ps[D:D + 1, :])
                bc_ps = ps.tile([64, 512], f32, tag="ps", name=f"bc_ps{h}_{qn}")
                nc.tensor.matmul(bc_ps, ones[:, 0:64],
                                 den, start=True, stop=True)
                bc_sb = small.tile([64, 512], f32, tag="bc", name=f"bc_sb{h}_{qn}")
                nc.vector.tensor_copy(bc_sb, bc_ps)
                for m in range(16):
                    nc.vector.tensor_mul(
                        y_sb[h][(m % 2) * 64:(m % 2) * 64 + 64, m // 2,
                                qn * 32:(qn + 1) * 32],
                        at_ps[0:D, m * 32:(m + 1) * 32],
                        bc_sb[:, m * 32:(m + 1) * 32])

        # ---- stage D: dense + residual + layernorm ----
        wd_sb = [wdpool.tile([128, HID], f32r, tag="wd", name=f"wd_sb{i}")
                 for i in range(8)]
        for ic in range(8):
            nc.sync.dma_start(wd_sb[ic], wd_d[ic * 128:(ic + 1) * 128, :])
        nstats = 2  # 1024 = 2 x 512 (BN_STATS_FMAX)
        for ci in range(4):
            qr = qrpool.tile([128, HID], f32, tag="qres", name=f"qr{ci}")
            nc.sync.dma_start(qr, qres_d[ci * 128:(ci + 1) * 128, :])
            x_t = xpool.tile([128, HID], f32, tag="x", name=f"x_t{ci}")
            for ob in range(2):
                dps = ps.tile([128, 512], f32, tag="ps", name=f"dps{ci}_{ob}")
                for pc in range(8):
                    nc.tensor.matmul(dps,
                                     y_sb[ci][:, pc, :],
                                     wd_sb[pc][:, ob * 512:(ob + 1) * 512],
                                     start=(pc == 0), stop=False)
                nc.tensor.matmul(dps, ones,
                                 bd_sb[:, ob * 512:(ob + 1) * 512],
                                 start=False, stop=True)
                nc.vector.tensor_add(x_t[:, ob * 512:(ob + 1) * 512], dps,
                                     qr[:, ob * 512:(ob + 1) * 512])

            stats = small.tile([128, nstats, nc.vector.BN_STATS_DIM], f32,
                               tag="st", name=f"stats{ci}")
            xg = x_t.rearrange("p (n d) -> p n d", n=nstats)
            for g in range(nstats):
                nc.vector.bn_stats(out=stats[:, g, :], in_=xg[:, g, :])
            mv = small.tile([128, nc.vector.BN_AGGR_DIM], f32, tag="mv",
                            name=f"mv{ci}")
            nc.vector.bn_aggr(out=mv, in_=stats)
            rstd = small.tile([128, 1], f32, tag="rstd", name=f"rstd{ci}")
            nc.scalar.activation(rstd, mv[:, 1:2], AF.Sqrt, bias=eps_sb, scale=1.0)
            nc.vector.reciprocal(rstd, rstd)
            nmr = small.tile([128, 1], f32, tag="nmr", name=f"nmr{ci}")
            nc.vector.tensor_scalar(out=nmr, in0=mv[:, 0:1], scalar1=rstd,
                                    scalar2=-1.0, op0=ALU.mult, op1=ALU.mult)
            nc.scalar.activation(x_t, x_t, AF.Identity, bias=nmr, scale=rstd)
            o_t = iopool.tile([128, HID], f32, tag="out", name=f"o_t{ci}")
            nc.vector.tensor_mul(o_t, x_t, gamma_sb)
            nc.vector.tensor_add(o_t, o_t, beta_sb)
            nc.sync.dma_start(out_d[ci * 128:(ci + 1) * 128, :], o_t)

    nc.compile()
    return nc


def _get_nc():
    if "nc" not in _cache:
        _cache["nc"] = _build_nc()
    return _cache["nc"]


def _host_prep(q, k, v, mask, wq, bq, wk, bk, wv, bv, wd, bd, gamma, beta):
    f = np.float32
    per_batch = []
    for b in range(B):
        kT = np.ascontiguousarray(np.asarray(k[b], f).T)
        vT = np.ascontiguousarray(np.asarray(v[b], f).T)
        qT = np.ascontiguousarray(np.asarray(q[b], f).T)
        mb = np.ascontiguousarray(
            ((1.0 - np.asarray(mask[b, 0, 0, :], f)) * np.float32(-1e9))
            .reshape(16, 128).T)
        per_batch.append((kT, vT, qT, mb))
    wk, wv, wq = (np.asarray(wk, f), np.asarray(wv, f), np.asarray(wq, f))
    bk, bv, bq = (np.asarray(bk, f), np.asarray(bv, f), np.asarray(bq, f))
    per_group = []
    for g in range(4):
        sl = slice(g * GW, (g + 1) * GW)
        per_group.append({
            "wkg": np.ascontiguousarray(wk[:, sl]),
            "wvg": np.ascontiguousarray(wv[:, sl]),
            "wqg": np.ascontiguousarray(wq[:, sl]),
            "bkg": np.ascontiguousarray(bk[sl].reshape(2, 128).T),
            "bqg": np.ascontiguousarray(bq[sl].reshape(2, 128).T),
            "bvg": bv[sl].reshape(1, GW).copy(),
        })
    shared = {
        "wd": np.ascontiguousarray(np.asarray(wd, f)),
        "bd_row": np.asarray(bd, f).reshape(1, HID).copy(),
        "gamma": np.asarray(gamma, f).reshape(1, HID).copy(),
        "beta": np.asarray(beta, f).reshape(1, HID).copy(),
        "ones_row": np.ones((1, 128), f),
        "ones_col": np.ones((128, GH, 1), f),
    }
    in_maps = []
    for c in range(NCORES):
        b, g = c // 4, c % 4
        kT, vT, qT, mb = per_batch[b]
        m = dict(shared)
        m.update(per_group[g])
        m["kT"] = kT
        m["vT"] = vT
        m["qT"] = qT
        m["qres"] = np.ascontiguousarray(np.asarray(q[b, g * QR:(g + 1) * QR, :], f))
        m["mbias"] = mb
        in_maps.append(m)
    return in_maps


def kernel(q, k, v, mask, wq, bq, wk, bk, wv, bv, wd, bd, gamma, beta):
    global last_results
    from concourse.bass_utils import run_bass_kernel_spmd

    in_maps = _host_prep(q, k, v, mask, wq, bq, wk, bk, wv, bv, wd, bd,
                         gamma, beta)
    nc = _get_nc()
    trace = bool(os.environ.get("KBENCH_TRACE"))
    res = run_bass_kernel_spmd(nc, in_maps, list(range(NCORES)), trace=trace)
    last_results = res
    out = np.empty((B, S, HID), np.float32)
    for c in range(NCORES):
        b, g = c // 4, c % 4
        out[b, g * QR:(g + 1) * QR, :] = res.results[c]["out"]
    return out
